# revision 93
# baseline (speedup 1.0000x reference)
"""Cubic B-spline FFD 3D upsampling kernel for Trainium2 (8 NeuronCores), v5.

v [4,3,44,52,44] f32 -> out [4,3,160,192,160] f32 via three separable stride-4
transposed convs (cubic B-spline, len 15) + crop. Sharded over output z: core c
consumes input planes [5c,5c+8), writes zo [20c,20c+20).

The z-axis expansion (44->160, the cheapest 1.3% of the FLOPs) is folded into
the host-side input prep: each core receives its z-expanded slab in fp16 (the
same precision the previous on-device z-pass produced). This trades a little
extra input-DMA bandwidth for the saturated vector/scalar engines' z time --
per-core HBM traffic actually rises (15.2 -> 17.2 MB); the device still does
all of the memory-dominant y/x expansion and output work.

Per-core pipeline (fp16 compute, f32 PSUM):
  L1 [128=(g@64, yi52pad64), (b6, zo20, xp64)]   <- 3 chunked DMAs
  y-pass on PE: per (g,zp) matmuls M=128: lhsT=L1[yi, (zo2,xp64)],
    rhs=Wy[52,192] -> py [128=(i_z@64, xi64), 192] f32, 2 matmuls/bank
  copy f32->f16 (DVE/ACT balanced; GPSIMD cannot read PSUM)
    -> L2b [128, (g2, zp10, yo192)]
  x-pass on PE: lhsT=L2b[64*i_z+(0:44), m-chunk128], rhs=Wx[44,160]
    -> px [128=m-chunk, 160] f32, 6 matmuls per 2-bank tile
  copy f32->f16 -> st [128, (q10=(i_z,u), jx480)]
  batched DMA st -> dev out [12, 10, 128, 480] f16: per (b,g) a q0:4 DMA and
    a deferred q4:10 DMA (issued a group later so its semaphore wait never
    head-of-line blocks the SP sequencer)
Host: permutation fixup + f16->f32.
"""

import os
import numpy as np

N_CORES = 8
ZIN, YIN, XIN = 44, 52, 44
ZOUT, YOUT, XOUT = 160, 192, 160
BC = 12
ZSH = ZOUT // N_CORES      # 20
ZISH = 8
XP = 64
N_WARM = 7


def _bspline_kernel():
    x = (np.arange(15) - 7) / 4.0
    t = np.abs(x)
    return np.where(
        t < 1.0, 2.0 / 3.0 + (0.5 * t - 1.0) * t**2,
        np.where(t < 2.0, ((2.0 - t) ** 3) / 6.0, 0.0)
    ).astype(np.float32)


_W = _bspline_kernel()


def _exp_mat(n_in, n_out):
    """M[i, o] = weight of control point i on (post-crop) output o."""
    M = np.zeros((n_in, n_out), dtype=np.float32)
    for o in range(n_out):
        for i in range(n_in):
            n = 4 * i - o + 3
            if 0 <= n < 15:
                M[i, o] = _W[n]
    return M


_NC_CACHE = {}

# measured marginal per-instruction ENGINE costs (ns) for copy load balancing
_COST_X = {"dve": 1125.0, "act": 985.0}
_COST_Y = {"dve": 525.0, "act": 463.0}
_PRELOAD = {"dve": 200.0, "act": 0.0}
# ready-time model: pipeline clock advances by each unit's avg cost / 2 eng
_READY0 = 2500.0


def _build_nc():
    import concourse.bacc as bacc
    import concourse.mybir as mybir
    from concourse.tile import TileContext

    FP32 = mybir.dt.float32
    FP16 = mybir.dt.float16

    nc = bacc.Bacc()
    v = nc.declare_dram_parameter("v", [2, YIN, 6 * ZSH * XP], FP16,
                                  isOutput=False)
    w = nc.declare_dram_parameter("w", [128, YOUT + XOUT], FP16,
                                  isOutput=False)
    out = nc.declare_dram_parameter("out", [BC, 10, 128, 480], FP16,
                                    isOutput=True)

    eng_busy = dict(_PRELOAD)
    pipe_t = [0.0]

    def pick(costs):
        """Finish-time greedy with a ready-time model for the copy input."""
        ready = _READY0 + pipe_t[0]
        pipe_t[0] += sum(costs.values()) / 6.0
        e = min(costs, key=lambda k: max(eng_busy[k], ready) + costs[k])
        eng_busy[e] = max(eng_busy[e], ready) + costs[e]
        return e

    def emit_copy(dst, src, costs):
        if pick(costs) == "dve":
            nc.vector.tensor_copy(out=dst, in_=src)
        else:
            nc.scalar.copy(dst, src)

    with TileContext(nc) as tc:
        with (
            tc.tile_pool(name="const", bufs=1) as cpool,
            tc.tile_pool(name="io", bufs=1) as iopool,
            tc.tile_pool(name="l2", bufs=2) as l2pool,
            tc.tile_pool(name="st", bufs=4) as stpool,
            tc.tile_pool(name="psy", bufs=2, space="PSUM") as psy,
            tc.tile_pool(name="psx", bufs=3, space="PSUM") as psx,
        ):
            # z-expanded input, x-pad cols pre-zeroed on the host; rows 52:64
            # per g are never read (y matmuls take [lo:lo+52]) so they are
            # neither sent nor zeroed. Chunked DMAs so the y-pass can start
            # after the first two b-slots land.
            L1 = iopool.tile([128, 6 * ZSH * XP], FP16)
            CB = ZSH * XP                               # cols per b-slot
            for c0, c1 in ((0, 2 * CB), (2 * CB, 4 * CB), (4 * CB, 6 * CB)):
                for g in range(2):
                    nc.sync.dma_start(out=L1[64 * g:64 * g + YIN, c0:c1],
                                      in_=v[g, :, c0:c1])

            wt = cpool.tile([128, YOUT + XOUT], FP16)
            nc.sync.dma_start(out=wt[:, :], in_=w[:, :])
            wyt = wt[:, 0:YOUT]
            wxt = wt[:, YOUT:YOUT + XOUT]

            warm = cpool.tile([128, 512], FP16)
            nc.vector.memset(warm[:, :], 0.0)
            # early one-time ACT table load, overlapping the input DMA
            acttiny = cpool.tile([128, 16], FP16)
            nc.scalar.copy(acttiny[:, :], warm[:, 0:16])

            pw = psx.tile([128, 1024], FP32, name="px")
            for _ in range(N_WARM):
                nc.tensor.matmul(pw[:, 0:512], lhsT=warm[0:52, 0:128],
                                 rhs=warm[0:52, :], start=True, stop=True,
                                 skip_group_check=True)

            outv = out  # [12, 10, 128, 480]
            L1q = L1.rearrange("p (b zp q) -> p b zp q", b=6, zp=10)

            def make_b(b):
                """Thunks for batch-slot b. x(b, g0) needs only y(b) t0..t4
                (likewise g1 / t5..t9), so the main loop half-shifts x
                against y instead of lagging a full b-slot."""
                L2b = l2pool.tile([128, 3840], FP16)
                stts = [stpool.tile([128, 10 * 480], FP16, name="stt")
                        for _ in range(2)]

                def ytile(t2, pool=None):
                    # b0's first phase has no x work in flight: borrow the
                    # idle psx slots so y isn't gated by psy's 2-slot loop
                    py = (pool.tile([128, 1024], FP32, name="px")[:, 0:512]
                          if pool is not None else psy.tile([128, 512], FP32))
                    for s in range(2):
                        p2 = 2 * t2 + s
                        g, zp = p2 // 10, p2 % 10
                        lo = 64 * g
                        nc.tensor.matmul(
                            py[:, 192 * s:192 * s + 192],
                            lhsT=L1q[lo:lo + YIN, b, zp, :],
                            rhs=wyt[lo:lo + YIN, :], start=True, stop=True)
                    dst = L2b[:, 384 * t2:384 * t2 + 384]
                    emit_copy(dst, py[:, 0:384], _COST_Y)

                def xtile(g, q6):
                    stt = stts[g]
                    bp = 6 * g + b
                    fine = (b == 0 and g == 0) or b == 5
                    px = psx.tile([128, 1024], FP32)
                    for s in range(6):
                        cg = 6 * q6 + s
                        iz, c = cg // 15, cg % 15
                        lo = 64 * iz
                        col = (s // 3) * 512 + (s % 3) * 160
                        nc.tensor.matmul(
                            px[:, col:col + 160],
                            lhsT=L2b[lo:lo + XIN,
                                     g * 1920 + 128 * c:
                                     g * 1920 + 128 * (c + 1)],
                            rhs=wxt[lo:lo + XIN, :],
                            start=True, stop=True)
                    src = px.rearrange("p (c q) -> p c q", c=2)[:, :, 0:480]
                    dst = stt[:, 960 * q6:960 * q6 + 960]
                    emit_copy(dst, src, _COST_X)
                    if fine:
                        # pipeline head/tail: small immediate DMAs to start
                        # the output stream early / shorten the drain
                        while q6 == 0 and pend_dma:
                            pend_dma.pop(0)()
                        nc.sync.dma_start(
                            out=outv[bp, 2 * q6:2 * q6 + 2]
                            .rearrange("q p f -> p q f"),
                            in_=stt[:, 960 * q6:960 * q6 + 960]
                            .rearrange("p (q f) -> p q f", q=2))
                    elif q6 in (1, 4):
                        # enqueue; issue deferred about a group later, when
                        # the copies are long done, so the semaphore wait
                        # never head-of-line blocks the SP sequencer
                        if q6 == 1:
                            def dma(bp=bp, stt=stt):
                                nc.sync.dma_start(
                                    out=outv[bp, 0:4]
                                    .rearrange("q p f -> p q f"),
                                    in_=stt[:, 0:1920]
                                    .rearrange("p (q f) -> p q f", q=4))
                        else:
                            def dma(bp=bp, stt=stt):
                                nc.sync.dma_start(
                                    out=outv[bp, 4:10]
                                    .rearrange("q p f -> p q f"),
                                    in_=stt[:, 1920:4800]
                                    .rearrange("p (q f) -> p q f", q=6))
                        pend_dma.append(dma)
                        if len(pend_dma) > 2:
                            pend_dma.pop(0)()

                ys = [lambda t2=t2, pool=None: ytile(t2, pool)
                      for t2 in range(10)]
                xs = [lambda g=g, q6=q6: xtile(g, q6)
                      for g in range(2) for q6 in range(5)]
                return ys, xs

            def run_phase(yy, xx):
                yi = xi2 = 0
                while yi < len(yy) or xi2 < len(xx):
                    if xi2 < len(xx):
                        xx[xi2](); xi2 += 1
                    if yi < len(yy):
                        yy[yi](); yi += 1

            pend_dma = []
            # merged start: y(b0) t0-4 (borrowed psx slots) interleaves with
            # y(b1) t0-4 (psy) -- both need only the first input chunk -- so
            # the copy engines ramp at double rate; one catch-up phase at b2
            # re-converges to the half-shifted steady state
            ys0, xs0 = make_b(0)
            ys1, xs1 = make_b(1)
            for i in range(5):
                ys0[i](pool=psx)
                ys1[i]()
            run_phase(ys0[5:10], xs0[0:5])      # y(b0) t5-t9 + x(b0, g0)
            run_phase(ys1[5:10], xs0[5:10])     # y(b1) t5-t9 + x(b0, g1)
            prevx = xs1[0:10]
            for b in range(2, 6):
                ys, xs = make_b(b)
                run_phase(ys[0:5], prevx)
                run_phase(ys[5:10], xs[0:5])
                prevx = xs[5:10]
            run_phase([], prevx)                # x(b5, g1)
            while pend_dma:
                pend_dma.pop(0)()

    nc.compile()
    return nc


def _get_nc():
    if "nc" not in _NC_CACHE:
        _NC_CACHE["nc"] = _build_nc()
    return _NC_CACHE["nc"]


_MZ = None


def _prep_inputs(v):
    """Full v [4,3,44,52,44] f32 -> per-core z-expanded input maps."""
    global _MZ
    f16 = np.float16
    v = np.asarray(v).astype(np.float32).reshape(BC, ZIN, YIN, XIN)

    w128 = np.zeros((128, YOUT + XOUT), dtype=np.float32)
    w128[0:YIN, 0:YOUT] = _exp_mat(YIN, YOUT)
    w128[64:64 + YIN, 0:YOUT] = w128[0:YIN, 0:YOUT]
    w128[0:XIN, YOUT:] = _exp_mat(XIN, XOUT)
    w128[64:64 + XIN, YOUT:] = w128[0:XIN, YOUT:]
    w_h = w128.astype(f16)

    if _MZ is None:
        _MZ = _exp_mat(ZIN, ZOUT)

    in_maps = []
    for c in range(N_CORES):
        slab = v[:, 5 * c:5 * c + ZISH]                    # [12, 8, 52, 44]
        mz = _MZ[5 * c:5 * c + ZISH, ZSH * c:ZSH * (c + 1)]   # [8, 20]
        zex = np.einsum("bzyx,zo->boyx", slab, mz)         # [12, 20, 52, 44]
        zex = zex.reshape(2, 6, ZSH, YIN, XIN).transpose(0, 3, 1, 2, 4)
        pad = np.zeros((2, YIN, 6, ZSH, XP), dtype=f16)
        pad[:, :, :, :, 0:XIN] = zex.astype(f16)
        in_maps.append({"v": pad.reshape(2, YIN, 6 * ZSH * XP), "w": w_h})
    return in_maps


def _assemble(results):
    """Per-core dev outputs [12, 10, 128, 480] f16 -> full f32 output."""
    out = np.empty((BC, ZOUT, YOUT, XOUT), dtype=np.float32)
    for c in range(N_CORES):
        dev = np.asarray(results[c]["out"])              # [12,10,128,480]
        dev = dev.reshape(BC, 2, 5, 128, 3, XOUT)
        dev = dev.transpose(0, 1, 2, 4, 3, 5)            # [12,2,5,3,128,160]
        dev = dev.reshape(BC, 2, 10, 192, XOUT)          # m -> (zp, yo)
        dev = dev.transpose(0, 2, 1, 3, 4)               # [12,10,2,192,160]
        blk = dev.reshape(BC, ZSH, YOUT, XOUT)
        out[:, ZSH * c:ZSH * (c + 1)] = blk.astype(np.float32)
    return out.reshape(4, 3, ZOUT, YOUT, XOUT)


def kernel(v):
    from concourse.bass_utils import run_bass_kernel_spmd

    in_maps = _prep_inputs(v)
    nc = _get_nc()
    res = run_bass_kernel_spmd(nc, in_maps, core_ids=list(range(N_CORES)))
    return _assemble(res.results)
